# revision 9
# baseline (speedup 1.0000x reference)
"""Trainium2 Bass kernel: per-row Euclidean projection onto
{p : 0 <= p <= PMAX, sum(p) <= BUDGET} (water-filling).

Full input raw_power (8192, 4096) f32 is sharded row-wise across 8 cores
(1024 rows each). Per core, rows live one-per-partition in 8 tiles of
[128, 4096]. The row threshold tau solving
    g(tau) = sum_i clip(x_i - tau, 0, PMAX) = BUDGET
is found with a safeguarded false-position (Illinois) iteration. Each
g-eval uses the numerically-stable split
    g(tau) = R(tau) - R(tau + PMAX),   R(s) = sum_i relu(x_i - s)
(relu sums stay small so fp32 sequential accumulation keeps ~1e-3
absolute error; clip-style sums at |x|~tau*N magnitude lose 100x more).
R passes run fused+accumulated: on ACT as activation(Relu, bias=-s) with
accum_out, on DVE as scalar_tensor_tensor((x - s) max 0) with accum_out.
The reference's 60 fp32 bisection steps converge to the same root, so
~9 evals reproduce its output to ~1e-5 relative error. Rows already
feasible (g(0) <= BUDGET) use tau = 0 == plain clip(x, 0, PMAX).
Per-row scalar state for all 8 tiles is batched in [128, 8] tiles so the
Illinois update chain costs ~20 tiny DVE ops per iteration total.
"""

import numpy as np

import concourse.bass as bass
import concourse.bacc as bacc
import concourse.mybir as mybir
from concourse.tile import TileContext
from concourse.bass_utils import run_bass_kernel_spmd

N_CORES = 8
ROWS = 8192
FD = 4096               # links per row
ROWS_PER_CORE = ROWS // N_CORES
P = 128                 # SBUF partitions
T = ROWS_PER_CORE // P  # 8 row-tiles per core
PMAX = 0.1
BUDGET = 100.0
K_ITERS = 8             # false-position iterations after the g(0) eval

F32 = mybir.dt.float32
Alu = mybir.AluOpType
Act = mybir.ActivationFunctionType
Axis = mybir.AxisListType


def _build_nc() -> bass.Bass:
    nc = bacc.Bacc("TRN2", target_bir_lowering=False)
    x_d = nc.dram_tensor("x", [ROWS_PER_CORE, FD], F32, kind="ExternalInput")
    y_d = nc.dram_tensor("y", [ROWS_PER_CORE, FD], F32, kind="ExternalOutput")
    xt = x_d[:, :].rearrange("(t p) d -> t p d", p=P)
    yt = y_d[:, :].rearrange("(t p) d -> t p d", p=P)

    with TileContext(nc) as tc:
        with (
            tc.tile_pool(name="data", bufs=1) as data,
            tc.tile_pool(name="scr", bufs=4) as scr,
            tc.tile_pool(name="st", bufs=1) as st,
        ):
            xs = []
            for t in range(T):
                x_tile = data.tile([P, FD], F32, tag=f"x{t}", name=f"x{t}")
                nc.sync.dma_start(x_tile[:, :], xt[t])
                xs.append(x_tile)

            def stile(nm):
                return st.tile([P, T], F32, tag=nm, name=nm)

            lo = stile("lo")
            hi = stile("hi")
            f_lo = stile("f_lo")
            f_hi = stile("f_hi")
            R1 = stile("R1")        # ACT accumulators: sum relu(x - tau)
            R2 = stile("R2")        # DVE accumulators: sum relu(x - tau - PMAX)
            ft = stile("ft")
            sv_i = st.tile([P, T], mybir.dt.int32, tag="sv_i", name="sv_i")
            sbar_i = st.tile([P, T], mybir.dt.int32, tag="sbar_i", name="sbar_i")
            last = stile("last")
            h = stile("h")
            d = stile("dnm")
            r = stile("rcp")
            w = stile("wdt")
            tv = stile("tv")        # current candidate tau per tile-column
            tp = stile("tp")        # tau + PMAX
            ntv = stile("ntv")      # -tau (ACT bias)
            infeas = stile("infeas")
            zcol = stile("zcol")    # [P, T] of zeros; column = broadcast src

            V = nc.vector
            A = nc.scalar

            V.memset(lo[:, :], 0.0)
            V.memset(f_hi[:, :], -BUDGET)
            V.memset(last[:, :], 0.0)
            V.memset(zcol[:, :], 0.0)

            def r_passes(k, thr_pos, thr_neg, thr_hi_pos, acc1, acc2):
                """One g-eval: R1[t] = sum relu(x - thr) on ACT,
                R2[t] = sum relu(x - thr - PMAX) on DVE, per tile t.
                thr_pos/thr_neg/thr_hi_pos: per-tile [P,1] APs or floats."""
                for t in range(T):
                    o1 = scr.tile([P, FD], F32, tag="scr", name=f"s{k}a{t}")
                    A.activation(
                        o1[:, :], xs[t][:, :], Act.Relu,
                        bias=thr_neg(t), scale=1.0,
                        accum_out=acc1[:, t : t + 1],
                    )
                    o2 = scr.tile([P, FD], F32, tag="scr", name=f"s{k}b{t}")
                    zb = zcol[:, t : t + 1].to_broadcast([P, FD])
                    V.scalar_tensor_tensor(
                        o2[:, :], xs[t][:, :], thr_hi_pos(t), zb,
                        op0=Alu.subtract, op1=Alu.max,
                        accum_out=acc2[:, t : t + 1],
                    )

            # rowmax -> hi (g(hi) = 0 exactly, so f_hi = -BUDGET with no eval)
            for t in range(T):
                V.reduce_max(hi[:, t : t + 1], xs[t][:, :], axis=Axis.X)

            # eval at tau=0: f_lo = g(0) - BUDGET
            r_passes("init", lambda t: 0.0, lambda t: 0.0, lambda t: PMAX, R1, R2)
            V.tensor_sub(f_lo[:, :], R1[:, :], R2[:, :])
            V.tensor_scalar(f_lo[:, :], f_lo[:, :], -BUDGET, None, op0=Alu.add)
            V.tensor_scalar(infeas[:, :], f_lo[:, :], 0.0, None, op0=Alu.is_gt)

            for k in range(K_ITERS + 1):
                # false-position candidate t = hi - f_hi*(hi-lo)/(f_hi-f_lo)
                V.tensor_sub(d[:, :], f_hi[:, :], f_lo[:, :])
                V.tensor_scalar(d[:, :], d[:, :], -1e-20, None, op0=Alu.min)
                V.reciprocal(r[:, :], d[:, :])
                V.tensor_sub(w[:, :], hi[:, :], lo[:, :])
                V.tensor_mul(w[:, :], w[:, :], f_hi[:, :])
                V.tensor_mul(w[:, :], w[:, :], r[:, :])
                V.tensor_sub(tv[:, :], hi[:, :], w[:, :])
                V.tensor_max(tv[:, :], tv[:, :], lo[:, :])
                V.tensor_tensor(tv[:, :], tv[:, :], hi[:, :], Alu.min)
                if k == K_ITERS:
                    break  # final candidate needs no evaluation
                V.tensor_scalar(tp[:, :], tv[:, :], PMAX, None, op0=Alu.add)
                V.tensor_scalar(ntv[:, :], tv[:, :], -1.0, None, op0=Alu.mult)

                r_passes(
                    k,
                    lambda t: tv[:, t : t + 1],
                    lambda t: ntv[:, t : t + 1],
                    lambda t: tp[:, t : t + 1],
                    R1, R2,
                )

                # f(t) = R1 - R2 - BUDGET
                V.tensor_sub(ft[:, :], R1[:, :], R2[:, :])
                V.tensor_scalar(ft[:, :], ft[:, :], -BUDGET, None, op0=Alu.add)
                V.tensor_scalar(sv_i[:, :], ft[:, :], 0.0, None, op0=Alu.is_gt)
                V.tensor_scalar(sbar_i[:, :], ft[:, :], 0.0, None, op0=Alu.is_le)
                # Illinois halving of the stale endpoint (harmless when the
                # endpoint is about to be replaced)
                V.tensor_scalar(h[:, :], last[:, :], 0.5, 0.5, op0=Alu.mult, op1=Alu.add)
                V.tensor_mul(f_lo[:, :], f_lo[:, :], h[:, :])
                V.tensor_scalar(h[:, :], last[:, :], -0.5, 1.0, op0=Alu.mult, op1=Alu.add)
                V.tensor_mul(f_hi[:, :], f_hi[:, :], h[:, :])
                V.copy_predicated(lo[:, :], sv_i[:, :], tv[:, :])
                V.copy_predicated(f_lo[:, :], sv_i[:, :], ft[:, :])
                V.copy_predicated(hi[:, :], sbar_i[:, :], tv[:, :])
                V.copy_predicated(f_hi[:, :], sbar_i[:, :], ft[:, :])
                V.tensor_copy(last[:, :], sv_i[:, :])

            # effective tau: 0 for feasible rows, tv otherwise
            V.tensor_mul(tv[:, :], tv[:, :], infeas[:, :])
            V.tensor_scalar(tp[:, :], tv[:, :], PMAX, None, op0=Alu.add)
            V.tensor_scalar(ntv[:, :], tv[:, :], -1.0, None, op0=Alu.mult)

            # out = min(max(x, tau), tau+PMAX) - tau, in place, then store
            for t in range(T):
                V.tensor_scalar(
                    xs[t][:, :], xs[t][:, :],
                    tv[:, t : t + 1], tp[:, t : t + 1],
                    op0=Alu.max, op1=Alu.min,
                )
                A.activation(
                    xs[t][:, :], xs[t][:, :], Act.Identity,
                    bias=ntv[:, t : t + 1], scale=1.0,
                )
                nc.gpsimd.dma_start(yt[t], xs[t][:, :])

    nc.finalize()
    return nc


_NC_CACHE = None


def _get_nc():
    global _NC_CACHE
    if _NC_CACHE is None:
        _NC_CACHE = _build_nc()
    return _NC_CACHE


def run(raw_power: np.ndarray, trace: bool = False):
    """Shard, run on 8 cores, gather. Returns (output, BassKernelResults)."""
    assert raw_power.shape == (ROWS, FD), raw_power.shape
    x = np.ascontiguousarray(raw_power, dtype=np.float32)
    shards = np.split(x, N_CORES, axis=0)
    nc = _get_nc()
    res = run_bass_kernel_spmd(
        nc,
        [{"x": s} for s in shards],
        core_ids=list(range(N_CORES)),
        trace=trace,
    )
    out = np.concatenate([r["y"] for r in res.results], axis=0)
    return out, res


def kernel(raw_power: np.ndarray) -> np.ndarray:
    out, _ = run(raw_power, trace=False)
    return out
